# revision 7
# baseline (speedup 1.0000x reference)
"""Trainium2 Bass kernel for the ConcreteAutoencoder ROM problem.

Strategy (hardcoded, self-contained):
  - Data-parallel over batch B=4096 across 8 cores (512 rows each).
  - Host pre-transposes each X/Y shard during sharding so the device
    contracts K=N=4096 with B on the moving (free) axis directly.
  - Device per core computes the transposed chain:
        Xt^T = (U^T X_s^T + 1)/2          (64, 512)
        Xh^T = phi^T Xt^T                 (256, 512)
        lib^T = [Xh^T; Xh^T^2; sin Xh^T]  (768, 512)
        samples = softmax((logits + gumbel)/temp)   (128, 768)
        A^T = samples lib^T               (128, 512)
        D^T = A_tilde Xt^T - Yt^T         (64, 512)
    and the B-contracted partials G0 = A^T A, ATB = A^T lhs_s,
    ATD = A^T D, dn = ||D||^2, yn = ||Yt||^2.
  - Host sums partials over cores and solves the 16 ridge systems
    (128x128) using the identity
        ||Xt At^T + A W - Yt||^2 = dn + 2 tr(W^T ATD) + tr(W^T G0 W)
    so no second pass over B is needed.
"""

import numpy as np

import concourse.bacc as bacc
import concourse.bass as bass
import concourse.mybir as mybir
import concourse.tile as tile
from concourse.bass_utils import run_bass_kernel_spmd

F32 = mybir.dt.float32
F32R = mybir.dt.float32r
AF = mybir.ActivationFunctionType

N_CORES = 8
B, N, R, NHAT, EOUT, EINP = 4096, 4096, 64, 256, 128, 768
BS = B // N_CORES   # 512 batch rows per core
KC = N // 128       # 32 contraction chunks
BC = BS // 128      # 4 batch sub-chunks of the shard
NLIB = EINP // 128  # 6 library chunks


def build_nc():
    from concourse.masks import make_identity

    nc = bacc.Bacc(None)
    xt = nc.dram_tensor("xt", [N, BS], F32, kind="ExternalInput")
    yt = nc.dram_tensor("yt", [N, BS], F32, kind="ExternalInput")
    lhs = nc.dram_tensor("lhs", [BS, R], F32, kind="ExternalInput")
    ul = nc.dram_tensor("ul", [N, R], F32, kind="ExternalInput")
    phim = nc.dram_tensor("phim", [R, NHAT], F32, kind="ExternalInput")
    atil = nc.dram_tensor("atil", [R, R], F32, kind="ExternalInput")
    logits = nc.dram_tensor("logits", [EOUT, EINP], F32, kind="ExternalInput")
    noise = nc.dram_tensor("noise", [EOUT, EINP], F32, kind="ExternalInput")
    itemp = nc.dram_tensor("itemp", [EOUT, 1], F32, kind="ExternalInput")

    g0_o = nc.dram_tensor("g0", [EOUT, EOUT], F32, kind="ExternalOutput")
    atb_o = nc.dram_tensor("atb", [EOUT, R], F32, kind="ExternalOutput")
    atd_o = nc.dram_tensor("atd", [EOUT, R], F32, kind="ExternalOutput")
    dn_o = nc.dram_tensor("dn", [R, 1], F32, kind="ExternalOutput")
    yn_o = nc.dram_tensor("yn", [R, 1], F32, kind="ExternalOutput")

    with tile.TileContext(nc) as tc:
        with (
            tc.tile_pool(name="const", bufs=1) as const,
            tc.tile_pool(name="xin", bufs=12) as xin,
            tc.tile_pool(name="ps_acc", bufs=4, space="PSUM") as ps_acc,
            tc.tile_pool(name="ps_t", bufs=3, space="PSUM") as ps_t,
        ):
            ident = const.tile([128, 128], F32)
            make_identity(nc, ident)

            u_sb = const.tile([128, KC, R], F32R)
            nc.gpsimd.dma_start(
                out=u_sb, in_=ul[:, :].rearrange("(c p) r -> p c r", p=128).bitcast(F32R))
            phi_sb = const.tile([R, NHAT], F32)
            nc.gpsimd.dma_start(out=phi_sb, in_=phim[:, :])
            at_sb = const.tile([R, R], F32)
            nc.gpsimd.dma_start(out=at_sb, in_=atil[:, :])
            lhs_sb = const.tile([128, BC, R], F32)
            nc.gpsimd.dma_start(out=lhs_sb, in_=lhs[:, :].rearrange("(c p) r -> p c r", p=128))
            logit_sb = const.tile([EOUT, EINP], F32)
            nc.gpsimd.dma_start(out=logit_sb, in_=logits[:, :])
            noise_sb = const.tile([EOUT, EINP], F32)
            nc.gpsimd.dma_start(out=noise_sb, in_=noise[:, :])
            itemp_sb = const.tile([EOUT, 1], F32)
            nc.gpsimd.dma_start(out=itemp_sb, in_=itemp[:, :])

            # gumbel-softmax selector: samples = softmax((logits - ln(-ln u)) / temp)
            t1 = const.tile([EOUT, EINP], F32)
            nc.scalar.activation(out=t1, in_=noise_sb, func=AF.Ln)
            nc.scalar.activation(out=t1, in_=t1, func=AF.Ln, scale=-1.0)
            z0 = const.tile([EOUT, EINP], F32)
            nc.vector.tensor_sub(z0, logit_sb, t1)
            rows = const.tile([EOUT, 1], F32)
            ez = const.tile([EOUT, EINP], F32)
            nc.scalar.activation(out=ez, in_=z0, func=AF.Exp, scale=itemp_sb, accum_out=rows)
            rinv = const.tile([EOUT, 1], F32)
            nc.vector.reciprocal(rinv, rows)
            samp = const.tile([EOUT, EINP], F32)
            nc.vector.tensor_scalar_mul(samp, ez, rinv)

            # samples^T chunks for use as matmul stationary operand
            sampT = const.tile([128, NLIB, EOUT], F32)
            for c in range(NLIB):
                pt = ps_t.tile([128, 128], F32, tag="pt")
                nc.tensor.transpose(pt, samp[:, c * 128:(c + 1) * 128], ident)
                nc.vector.tensor_copy(sampT[:, c], pt)

            # A_tilde^T (stationary operand for D^T matmul)
            atT = const.tile([R, R], F32)
            ptA = ps_t.tile([128, 128], F32, tag="pt")
            nc.tensor.transpose(ptA[:R, :R], at_sb, ident[:R, :R])
            nc.vector.tensor_copy(atT, ptA[:R, :R])

            # main contraction: Xt^T/Yt^T = U^T @ X_s^T (accumulate over 32 k-chunks)
            xacc = ps_acc.tile([R, BS], F32, tag="acc")
            yacc = ps_acc.tile([R, BS], F32, tag="acc")
            for c in range(KC):
                xtile = xin.tile([128, BS], F32R, tag="xin")
                nc.sync.dma_start(
                    out=xtile, in_=xt[c * 128:(c + 1) * 128, :].bitcast(F32R))
                nc.tensor.matmul(
                    xacc, u_sb[:, c], xtile[:],
                    start=(c == 0), stop=(c == KC - 1))
            for c in range(KC):
                ytile = xin.tile([128, BS], F32R, tag="xin")
                nc.scalar.dma_start(
                    out=ytile, in_=yt[c * 128:(c + 1) * 128, :].bitcast(F32R))
                nc.tensor.matmul(
                    yacc, u_sb[:, c], ytile[:],
                    start=(c == 0), stop=(c == KC - 1))

            # affine (x + 1)/2 applied during PSUM->SBUF copy
            xt_T = const.tile([R, BS], F32)
            nc.scalar.activation(out=xt_T, in_=xacc, func=AF.Copy, scale=0.5, bias=0.5)
            yt_T = const.tile([R, BS], F32)
            nc.scalar.activation(out=yt_T, in_=yacc, func=AF.Copy, scale=0.5, bias=0.5)

            # yn = ||Yt||^2 (per-partition partial sums)
            sq_y = const.tile([R, BS], F32)
            yn_sb = const.tile([R, 1], F32)
            nc.scalar.activation(out=sq_y, in_=yt_T, func=AF.Square, accum_out=yn_sb)

            # Xh^T = phi^T @ Xt^T, then library [id, square, sin]
            libT = const.tile([128, NLIB, BS], F32)
            for c in range(2):
                xh_ps = ps_acc.tile([128, BS], F32, tag="acc")
                nc.tensor.matmul(xh_ps, phi_sb[:, c * 128:(c + 1) * 128], xt_T[:],
                                 start=True, stop=True)
                nc.vector.tensor_copy(libT[:, c], xh_ps)
                nc.vector.tensor_mul(libT[:, 2 + c], libT[:, c], libT[:, c])
                nc.scalar.activation(out=libT[:, 4 + c], in_=xh_ps, func=AF.Sin)

            # A^T = samples @ lib^T (accumulate over 6 library chunks)
            a_ps = ps_acc.tile([128, BS], F32, tag="acc")
            for c in range(NLIB):
                nc.tensor.matmul(a_ps, sampT[:, c], libT[:, c],
                                 start=(c == 0), stop=(c == NLIB - 1))
            aT = const.tile([128, BS], F32)
            nc.vector.tensor_copy(aT, a_ps)

            # D^T = A_tilde @ Xt^T - Yt^T
            d_ps = ps_acc.tile([R, BS], F32, tag="acc")
            nc.tensor.matmul(d_ps, atT[:], xt_T[:], start=True, stop=True)
            dT = const.tile([R, BS], F32)
            nc.vector.tensor_sub(dT, d_ps, yt_T)
            sq_d = const.tile([R, BS], F32)
            dn_sb = const.tile([R, 1], F32)
            nc.scalar.activation(out=sq_d, in_=dT, func=AF.Square, accum_out=dn_sb)

            # transpose A^T and D^T back to batch-major for the B-contractions
            a_nat = const.tile([128, BC, 128], F32)
            d_nat = const.tile([128, BC, R], F32)
            for bc in range(BC):
                pa = ps_t.tile([128, 128], F32, tag="pt")
                nc.tensor.transpose(pa, aT[:, bc * 128:(bc + 1) * 128], ident)
                nc.vector.tensor_copy(a_nat[:, bc], pa)
                pd = ps_t.tile([128, 128], F32, tag="pt")
                nc.tensor.transpose(pd[:, :R], dT[:, bc * 128:(bc + 1) * 128],
                                    ident[:R, :R])
                nc.vector.tensor_copy(d_nat[:, bc], pd[:, :R])

            # G0 = A^T A, ATB = A^T lhs, ATD = A^T D  (full fp32)
            g_ps = ps_acc.tile([128, 128], F32, tag="acc")
            for bc in range(BC):
                nc.tensor.matmul(g_ps, a_nat[:, bc], a_nat[:, bc],
                                 start=(bc == 0), stop=(bc == BC - 1))
            g_sb = const.tile([128, 128], F32)
            nc.vector.tensor_copy(g_sb, g_ps)
            nc.sync.dma_start(out=g0_o[:, :], in_=g_sb)

            atb_ps = ps_acc.tile([128, R], F32, tag="acc")
            for bc in range(BC):
                nc.tensor.matmul(atb_ps, a_nat[:, bc], lhs_sb[:, bc],
                                 start=(bc == 0), stop=(bc == BC - 1))
            atb_sb = const.tile([128, R], F32)
            nc.vector.tensor_copy(atb_sb, atb_ps)
            nc.sync.dma_start(out=atb_o[:, :], in_=atb_sb)

            atd_ps = ps_acc.tile([128, R], F32, tag="acc")
            for bc in range(BC):
                nc.tensor.matmul(atd_ps, a_nat[:, bc], d_nat[:, bc],
                                 start=(bc == 0), stop=(bc == BC - 1))
            atd_sb = const.tile([128, R], F32)
            nc.vector.tensor_copy(atd_sb, atd_ps)
            nc.sync.dma_start(out=atd_o[:, :], in_=atd_sb)

            nc.sync.dma_start(out=dn_o[:, :], in_=dn_sb)
            nc.sync.dma_start(out=yn_o[:, :], in_=yn_sb)

    nc.compile()
    return nc


_cache = {}


def kernel(X_batch_t, Y_batch_t, lhs_mat_batch_t, temperature_spt, logits,
           uniform_noise, U_l, phi_mat, A_tilde, lam_vec,
           _trace=False, _trace_cores=None):
    if "nc" not in _cache:
        _cache["nc"] = build_nc()
    nc = _cache["nc"]

    X = np.asarray(X_batch_t, dtype=np.float32)
    Y = np.asarray(Y_batch_t, dtype=np.float32)
    lhs = np.asarray(lhs_mat_batch_t, dtype=np.float32)
    U = np.ascontiguousarray(np.asarray(U_l, dtype=np.float32))
    phim = np.ascontiguousarray(np.asarray(phi_mat, dtype=np.float32))
    atil = np.ascontiguousarray(np.asarray(A_tilde, dtype=np.float32))
    logits_np = np.ascontiguousarray(np.asarray(logits, dtype=np.float32))
    noise_np = np.ascontiguousarray(np.asarray(uniform_noise, dtype=np.float32))
    tval = np.float32(np.asarray(temperature_spt).reshape(-1)[0])
    temp = np.maximum(np.float32(0.1), tval * np.float32(0.99))
    itemp = np.full((EOUT, 1), np.float32(1.0) / temp, dtype=np.float32)

    in_maps = []
    for c in range(N_CORES):
        sl = slice(c * BS, (c + 1) * BS)
        in_maps.append({
            "xt": np.ascontiguousarray(X[sl].T),
            "yt": np.ascontiguousarray(Y[sl].T),
            "lhs": np.ascontiguousarray(lhs[sl]),
            "ul": U, "phim": phim, "atil": atil,
            "logits": logits_np, "noise": noise_np, "itemp": itemp,
        })

    kw = {}
    if _trace:
        kw["trace"] = True
        kw["trace_cores"] = _trace_cores or list(range(N_CORES))
    res = run_bass_kernel_spmd(nc, in_maps, core_ids=list(range(N_CORES)), **kw)
    kernel.last = res
    results = res.results

    G0 = np.zeros((EOUT, EOUT), np.float64)
    ATB = np.zeros((EOUT, R), np.float64)
    ATD = np.zeros((EOUT, R), np.float64)
    dn = 0.0
    yn = 0.0
    for r in results:
        G0 += r["g0"].astype(np.float64)
        ATB += r["atb"].astype(np.float64)
        ATD += r["atd"].astype(np.float64)
        dn += float(r["dn"].astype(np.float64).sum())
        yn += float(r["yn"].astype(np.float64).sum())

    lamv = np.asarray(lam_vec, dtype=np.float64).reshape(-1)
    eye = np.eye(EOUT)
    errs = np.empty(len(lamv))
    Ws = []
    for i, lam in enumerate(lamv):
        W = np.linalg.solve(G0 + lam * eye, ATB)
        num = dn + 2.0 * float(np.sum(W * ATD)) + float(np.sum(W * (G0 @ W)))
        errs[i] = np.sqrt(max(num, 0.0) / yn)
        Ws.append(W)
    iopt = int(np.nanargmin(errs))
    phi_bar = np.ascontiguousarray(Ws[iopt].T).astype(np.float32)
    selected_idx = np.argmax(logits_np, axis=-1).astype(np.int32)
    return (selected_idx, np.float32(errs[iopt]), phi_bar, np.float32(temp))


kernel.last = None


# revision 10
# speedup vs baseline: 1.0516x; 1.0516x over previous
"""Trainium2 Bass kernel for the ConcreteAutoencoder ROM problem.

Strategy (hardcoded, self-contained):
  - Data-parallel over batch B=4096 across 8 cores (512 rows each).
  - Host pre-transposes each X/Y shard during sharding so the device
    contracts K=N=4096 with B on the moving (free) axis directly.
  - Device per core computes the transposed chain:
        Xt^T = (U^T X_s^T + 1)/2          (64, 512)
        Xh^T = phi^T Xt^T                 (256, 512)
        lib^T = [Xh^T; Xh^T^2; sin Xh^T]  (768, 512)
        samples = softmax((logits + gumbel)/temp)   (128, 768)
        A^T = samples lib^T               (128, 512)
        D^T = A_tilde Xt^T - Yt^T         (64, 512)
    and the B-contracted partials G0 = A^T A, ATB = A^T lhs_s,
    ATD = A^T D, dn = ||D||^2, yn = ||Yt||^2.
  - Host sums partials over cores and solves the 16 ridge systems
    (128x128) using the identity
        ||Xt At^T + A W - Yt||^2 = dn + 2 tr(W^T ATD) + tr(W^T G0 W)
    so no second pass over B is needed.
"""

import numpy as np

import concourse.bacc as bacc
import concourse.bass as bass
import concourse.mybir as mybir
import concourse.tile as tile
from concourse.bass_utils import run_bass_kernel_spmd

F32 = mybir.dt.float32
F32R = mybir.dt.float32r
AF = mybir.ActivationFunctionType

N_CORES = 8
B, N, R, NHAT, EOUT, EINP = 4096, 4096, 64, 256, 128, 768
BS = B // N_CORES   # 512 batch rows per core
KC = N // 128       # 32 contraction chunks
BC = BS // 128      # 4 batch sub-chunks of the shard
NLIB = EINP // 128  # 6 library chunks


def build_nc():
    from concourse.masks import make_identity

    nc = bacc.Bacc(None)
    xt = nc.dram_tensor("xt", [N, BS], F32, kind="ExternalInput")
    yt = nc.dram_tensor("yt", [N, BS], F32, kind="ExternalInput")
    lhs = nc.dram_tensor("lhs", [BS, R], F32, kind="ExternalInput")
    ul = nc.dram_tensor("ul", [N, R], F32, kind="ExternalInput")
    phim = nc.dram_tensor("phim", [R, NHAT], F32, kind="ExternalInput")
    atil = nc.dram_tensor("atil", [R, R], F32, kind="ExternalInput")
    logits = nc.dram_tensor("logits", [EOUT, EINP], F32, kind="ExternalInput")
    noise = nc.dram_tensor("noise", [EOUT, EINP], F32, kind="ExternalInput")
    itemp = nc.dram_tensor("itemp", [EOUT, 1], F32, kind="ExternalInput")

    g0_o = nc.dram_tensor("g0", [EOUT, EOUT], F32, kind="ExternalOutput")
    atb_o = nc.dram_tensor("atb", [EOUT, R], F32, kind="ExternalOutput")
    atd_o = nc.dram_tensor("atd", [EOUT, R], F32, kind="ExternalOutput")
    dn_o = nc.dram_tensor("dn", [R, 1], F32, kind="ExternalOutput")
    yn_o = nc.dram_tensor("yn", [R, 1], F32, kind="ExternalOutput")

    with tile.TileContext(nc) as tc:
        with (
            tc.tile_pool(name="const", bufs=1) as const,
            tc.tile_pool(name="xin", bufs=32) as xin,
            tc.tile_pool(name="ps_acc", bufs=4, space="PSUM") as ps_acc,
            tc.tile_pool(name="ps_t", bufs=3, space="PSUM") as ps_t,
        ):
            ident = const.tile([128, 128], F32)
            make_identity(nc, ident)

            u_sb = const.tile([128, KC, R], F32R)
            nc.sync.dma_start(
                out=u_sb, in_=ul[:, :].rearrange("(c p) r -> p c r", p=128).bitcast(F32R))
            noise_sb = const.tile([EOUT, EINP], F32)
            nc.scalar.dma_start(out=noise_sb, in_=noise[:, :])
            logit_sb = const.tile([EOUT, EINP], F32)
            nc.scalar.dma_start(out=logit_sb, in_=logits[:, :])
            itemp_sb = const.tile([EOUT, 1], F32)
            nc.scalar.dma_start(out=itemp_sb, in_=itemp[:, :])
            phi_sb = const.tile([R, NHAT], F32R)
            nc.scalar.dma_start(out=phi_sb, in_=phim[:, :].bitcast(F32R))
            at_sb = const.tile([R, R], F32)
            nc.scalar.dma_start(out=at_sb, in_=atil[:, :])
            lhs_sb = const.tile([128, BC, R], F32)
            nc.scalar.dma_start(out=lhs_sb, in_=lhs[:, :].rearrange("(c p) r -> p c r", p=128))

            # gumbel-softmax selector: samples = softmax((logits - ln(-ln u)) / temp)
            t1 = const.tile([EOUT, EINP], F32)
            nc.scalar.activation(out=t1, in_=noise_sb, func=AF.Ln)
            nc.scalar.activation(out=t1, in_=t1, func=AF.Ln, scale=-1.0)
            z0 = const.tile([EOUT, EINP], F32)
            nc.vector.tensor_sub(z0, logit_sb, t1)
            rows = const.tile([EOUT, 1], F32)
            ez = const.tile([EOUT, EINP], F32)
            nc.scalar.activation(out=ez, in_=z0, func=AF.Exp, scale=itemp_sb, accum_out=rows)
            rinv = const.tile([EOUT, 1], F32)
            nc.vector.reciprocal(rinv, rows)
            samp = const.tile([EOUT, EINP], F32)
            nc.vector.tensor_scalar_mul(samp, ez, rinv)

            # samples^T chunks for use as matmul stationary operand
            sampT = const.tile([128, NLIB, EOUT], F32R)
            for c in range(NLIB):
                pt = ps_t.tile([128, 128], F32, tag="pt")
                nc.tensor.transpose(pt, samp[:, c * 128:(c + 1) * 128], ident)
                nc.vector.tensor_copy(sampT[:, c], pt)

            # A_tilde^T (stationary operand for D^T matmul)
            atT = const.tile([R, R], F32R)
            ptA = ps_t.tile([128, 128], F32, tag="pt")
            nc.tensor.transpose(ptA[:R, :R], at_sb, ident[:R, :R])
            nc.vector.tensor_copy(atT, ptA[:R, :R])

            # main contraction: Xt^T/Yt^T = U^T @ X_s^T (accumulate over 32 k-chunks)
            xacc = ps_acc.tile([R, BS], F32, tag="acc")
            yacc = ps_acc.tile([R, BS], F32, tag="acc")
            for c in range(KC):
                xtile = xin.tile([128, BS], F32R, tag="xin")
                nc.sync.dma_start(
                    out=xtile, in_=xt[c * 128:(c + 1) * 128, :].bitcast(F32R))
                nc.tensor.matmul(
                    xacc, u_sb[:, c], xtile[:],
                    start=(c == 0), stop=(c == KC - 1))
            for c in range(KC):
                ytile = xin.tile([128, BS], F32R, tag="xin")
                nc.scalar.dma_start(
                    out=ytile, in_=yt[c * 128:(c + 1) * 128, :].bitcast(F32R))
                nc.tensor.matmul(
                    yacc, u_sb[:, c], ytile[:],
                    start=(c == 0), stop=(c == KC - 1))

            # affine (x + 1)/2 applied during PSUM->SBUF copy
            xt_T = const.tile([R, BS], F32R)
            nc.scalar.activation(out=xt_T, in_=xacc, func=AF.Copy, scale=0.5, bias=0.5)
            yt_T = const.tile([R, BS], F32)
            nc.scalar.activation(out=yt_T, in_=yacc, func=AF.Copy, scale=0.5, bias=0.5)

            # yn = ||Yt||^2 (per-partition partial sums)
            sq_y = const.tile([R, BS], F32)
            yn_sb = const.tile([R, 1], F32)
            nc.scalar.activation(out=sq_y, in_=yt_T, func=AF.Square, accum_out=yn_sb)

            # Xh^T = phi^T @ Xt^T, then library [id, square, sin]
            libT = const.tile([128, NLIB, BS], F32R)
            for c in range(2):
                xh_ps = ps_acc.tile([128, BS], F32, tag="acc")
                nc.tensor.matmul(xh_ps, phi_sb[:, c * 128:(c + 1) * 128], xt_T[:],
                                 start=True, stop=True)
                nc.vector.tensor_copy(libT[:, c], xh_ps)
                nc.vector.tensor_mul(libT[:, 2 + c], libT[:, c], libT[:, c])
                nc.scalar.activation(out=libT[:, 4 + c], in_=xh_ps, func=AF.Sin)

            # A^T = samples @ lib^T (accumulate over 6 library chunks)
            a_ps = ps_acc.tile([128, BS], F32, tag="acc")
            for c in range(NLIB):
                nc.tensor.matmul(a_ps, sampT[:, c], libT[:, c],
                                 start=(c == 0), stop=(c == NLIB - 1))
            aT = const.tile([128, BS], F32)
            nc.vector.tensor_copy(aT, a_ps)

            # D^T = A_tilde @ Xt^T - Yt^T
            d_ps = ps_acc.tile([R, BS], F32, tag="acc")
            nc.tensor.matmul(d_ps, atT[:], xt_T[:], start=True, stop=True)
            dT = const.tile([R, BS], F32)
            nc.vector.tensor_sub(dT, d_ps, yt_T)
            sq_d = const.tile([R, BS], F32)
            dn_sb = const.tile([R, 1], F32)
            nc.scalar.activation(out=sq_d, in_=dT, func=AF.Square, accum_out=dn_sb)

            # transpose A^T and D^T back to batch-major for the B-contractions
            a_nat = const.tile([128, BC, 128], F32)
            d_nat = const.tile([128, BC, R], F32)
            for bc in range(BC):
                pa = ps_t.tile([128, 128], F32, tag="pt")
                nc.tensor.transpose(pa, aT[:, bc * 128:(bc + 1) * 128], ident)
                nc.vector.tensor_copy(a_nat[:, bc], pa)
                pd = ps_t.tile([128, 128], F32, tag="pt")
                nc.tensor.transpose(pd[:, :R], dT[:, bc * 128:(bc + 1) * 128],
                                    ident[:R, :R])
                nc.vector.tensor_copy(d_nat[:, bc], pd[:, :R])

            # G0 = A^T A, ATB = A^T lhs, ATD = A^T D  (full fp32)
            g_ps = ps_acc.tile([128, 128], F32, tag="acc")
            for bc in range(BC):
                nc.tensor.matmul(g_ps, a_nat[:, bc], a_nat[:, bc],
                                 start=(bc == 0), stop=(bc == BC - 1))
            g_sb = const.tile([128, 128], F32)
            nc.vector.tensor_copy(g_sb, g_ps)
            nc.sync.dma_start(out=g0_o[:, :], in_=g_sb)

            atb_ps = ps_acc.tile([128, R], F32, tag="acc")
            for bc in range(BC):
                nc.tensor.matmul(atb_ps, a_nat[:, bc], lhs_sb[:, bc],
                                 start=(bc == 0), stop=(bc == BC - 1))
            atb_sb = const.tile([128, R], F32)
            nc.vector.tensor_copy(atb_sb, atb_ps)
            nc.sync.dma_start(out=atb_o[:, :], in_=atb_sb)

            atd_ps = ps_acc.tile([128, R], F32, tag="acc")
            for bc in range(BC):
                nc.tensor.matmul(atd_ps, a_nat[:, bc], d_nat[:, bc],
                                 start=(bc == 0), stop=(bc == BC - 1))
            atd_sb = const.tile([128, R], F32)
            nc.vector.tensor_copy(atd_sb, atd_ps)
            nc.sync.dma_start(out=atd_o[:, :], in_=atd_sb)

            nc.sync.dma_start(out=dn_o[:, :], in_=dn_sb)
            nc.sync.dma_start(out=yn_o[:, :], in_=yn_sb)

    nc.compile()
    return nc


_cache = {}


def kernel(X_batch_t, Y_batch_t, lhs_mat_batch_t, temperature_spt, logits,
           uniform_noise, U_l, phi_mat, A_tilde, lam_vec,
           _trace=False, _trace_cores=None):
    if "nc" not in _cache:
        _cache["nc"] = build_nc()
    nc = _cache["nc"]

    X = np.asarray(X_batch_t, dtype=np.float32)
    Y = np.asarray(Y_batch_t, dtype=np.float32)
    lhs = np.asarray(lhs_mat_batch_t, dtype=np.float32)
    U = np.ascontiguousarray(np.asarray(U_l, dtype=np.float32))
    phim = np.ascontiguousarray(np.asarray(phi_mat, dtype=np.float32))
    atil = np.ascontiguousarray(np.asarray(A_tilde, dtype=np.float32))
    logits_np = np.ascontiguousarray(np.asarray(logits, dtype=np.float32))
    noise_np = np.ascontiguousarray(np.asarray(uniform_noise, dtype=np.float32))
    tval = np.float32(np.asarray(temperature_spt).reshape(-1)[0])
    temp = np.maximum(np.float32(0.1), tval * np.float32(0.99))
    itemp = np.full((EOUT, 1), np.float32(1.0) / temp, dtype=np.float32)

    in_maps = []
    for c in range(N_CORES):
        sl = slice(c * BS, (c + 1) * BS)
        in_maps.append({
            "xt": np.ascontiguousarray(X[sl].T),
            "yt": np.ascontiguousarray(Y[sl].T),
            "lhs": np.ascontiguousarray(lhs[sl]),
            "ul": U, "phim": phim, "atil": atil,
            "logits": logits_np, "noise": noise_np, "itemp": itemp,
        })

    kw = {}
    if _trace:
        kw["trace"] = True
        kw["trace_cores"] = _trace_cores or list(range(N_CORES))
    res = run_bass_kernel_spmd(nc, in_maps, core_ids=list(range(N_CORES)), **kw)
    kernel.last = res
    results = res.results

    G0 = np.zeros((EOUT, EOUT), np.float64)
    ATB = np.zeros((EOUT, R), np.float64)
    ATD = np.zeros((EOUT, R), np.float64)
    dn = 0.0
    yn = 0.0
    for r in results:
        G0 += r["g0"].astype(np.float64)
        ATB += r["atb"].astype(np.float64)
        ATD += r["atd"].astype(np.float64)
        dn += float(r["dn"].astype(np.float64).sum())
        yn += float(r["yn"].astype(np.float64).sum())

    lamv = np.asarray(lam_vec, dtype=np.float64).reshape(-1)
    eye = np.eye(EOUT)
    errs = np.empty(len(lamv))
    Ws = []
    for i, lam in enumerate(lamv):
        W = np.linalg.solve(G0 + lam * eye, ATB)
        num = dn + 2.0 * float(np.sum(W * ATD)) + float(np.sum(W * (G0 @ W)))
        errs[i] = np.sqrt(max(num, 0.0) / yn)
        Ws.append(W)
    iopt = int(np.nanargmin(errs))
    phi_bar = np.ascontiguousarray(Ws[iopt].T).astype(np.float32)
    selected_idx = np.argmax(logits_np, axis=-1).astype(np.int32)
    return (selected_idx, np.float32(errs[iopt]), phi_bar, np.float32(temp))


kernel.last = None


# revision 11
# speedup vs baseline: 1.1123x; 1.0578x over previous
"""Trainium2 Bass kernel for the ConcreteAutoencoder ROM problem.

Strategy (hardcoded, self-contained):
  - Data-parallel over batch B=4096 across 8 cores (512 rows each).
  - Host pre-transposes each X/Y shard during sharding so the device
    contracts K=N=4096 with B on the moving (free) axis directly.
  - Device per core computes the transposed chain:
        Xt^T = (U^T X_s^T + 1)/2          (64, 512)
        Xh^T = phi^T Xt^T                 (256, 512)
        lib^T = [Xh^T; Xh^T^2; sin Xh^T]  (768, 512)
        samples = softmax((logits + gumbel)/temp)   (128, 768)
        A^T = samples lib^T               (128, 512)
        D^T = A_tilde Xt^T - Yt^T         (64, 512)
    and the B-contracted partials G0 = A^T A, ATB = A^T lhs_s,
    ATD = A^T D, dn = ||D||^2, yn = ||Yt||^2.
  - Host sums partials over cores and solves the 16 ridge systems
    (128x128) using the identity
        ||Xt At^T + A W - Yt||^2 = dn + 2 tr(W^T ATD) + tr(W^T G0 W)
    so no second pass over B is needed.
"""

import numpy as np

import concourse.bacc as bacc
import concourse.bass as bass
import concourse.mybir as mybir
import concourse.tile as tile
from concourse.bass_utils import run_bass_kernel_spmd

F32 = mybir.dt.float32
F32R = mybir.dt.float32r
AF = mybir.ActivationFunctionType

N_CORES = 8
B, N, R, NHAT, EOUT, EINP = 4096, 4096, 64, 256, 128, 768
BS = B // N_CORES   # 512 batch rows per core
KC = N // 128       # 32 contraction chunks
BC = BS // 128      # 4 batch sub-chunks of the shard
NLIB = EINP // 128  # 6 library chunks


def build_nc():
    from concourse.masks import make_identity

    nc = bacc.Bacc(None)
    xt = nc.dram_tensor("xt", [N, BS], F32, kind="ExternalInput")
    yt = nc.dram_tensor("yt", [N, BS], F32, kind="ExternalInput")
    lhs = nc.dram_tensor("lhs", [128, BC * R], F32, kind="ExternalInput")
    ul = nc.dram_tensor("ul", [128, KC * R], F32, kind="ExternalInput")
    phim = nc.dram_tensor("phim", [R, NHAT], F32, kind="ExternalInput")
    atil = nc.dram_tensor("atil", [R, R], F32, kind="ExternalInput")
    logits = nc.dram_tensor("logits", [EOUT, EINP], F32, kind="ExternalInput")
    noise = nc.dram_tensor("noise", [EOUT, EINP], F32, kind="ExternalInput")
    itemp = nc.dram_tensor("itemp", [EOUT, 1], F32, kind="ExternalInput")

    g0_o = nc.dram_tensor("g0", [EOUT, EOUT], F32, kind="ExternalOutput")
    atb_o = nc.dram_tensor("atb", [EOUT, R], F32, kind="ExternalOutput")
    atd_o = nc.dram_tensor("atd", [EOUT, R], F32, kind="ExternalOutput")
    dn_o = nc.dram_tensor("dn", [R, 1], F32, kind="ExternalOutput")
    yn_o = nc.dram_tensor("yn", [R, 1], F32, kind="ExternalOutput")

    with tile.TileContext(nc) as tc:
        with (
            tc.tile_pool(name="const", bufs=1) as const,
            tc.tile_pool(name="xin", bufs=32) as xin,
            tc.tile_pool(name="ps_acc", bufs=4, space="PSUM") as ps_acc,
            tc.tile_pool(name="ps_t", bufs=3, space="PSUM") as ps_t,
        ):
            ident = const.tile([128, 128], F32)
            make_identity(nc, ident)

            # U pre-reshaped on host to partition-major: one dense DMA
            u_sb = const.tile([128, KC, R], F32R)
            nc.sync.dma_start(out=u_sb, in_=ul[:, :].bitcast(F32R))
            phi_sb = const.tile([R, NHAT], F32R)
            nc.scalar.dma_start(out=phi_sb, in_=phim[:, :].bitcast(F32R))
            lhs_sb = const.tile([128, BC, R], F32)
            nc.scalar.dma_start(out=lhs_sb, in_=lhs[:, :])
            # small consts on the SWDGE path (parallel to the main queues)
            noise_sb = const.tile([EOUT, EINP], F32)
            nc.gpsimd.dma_start(out=noise_sb, in_=noise[:, :])
            logit_sb = const.tile([EOUT, EINP], F32)
            nc.gpsimd.dma_start(out=logit_sb, in_=logits[:, :])
            itemp_sb = const.tile([EOUT, 1], F32)
            nc.gpsimd.dma_start(out=itemp_sb, in_=itemp[:, :])
            at_sb = const.tile([R, R], F32)
            nc.gpsimd.dma_start(out=at_sb, in_=atil[:, :])

            # gumbel-softmax selector: samples = softmax((logits - ln(-ln u)) / temp)
            t1 = const.tile([EOUT, EINP], F32)
            nc.scalar.activation(out=t1, in_=noise_sb, func=AF.Ln)
            nc.scalar.activation(out=t1, in_=t1, func=AF.Ln, scale=-1.0)
            z0 = const.tile([EOUT, EINP], F32)
            nc.vector.tensor_sub(z0, logit_sb, t1)
            rows = const.tile([EOUT, 1], F32)
            ez = const.tile([EOUT, EINP], F32)
            nc.scalar.activation(out=ez, in_=z0, func=AF.Exp, scale=itemp_sb, accum_out=rows)
            rinv = const.tile([EOUT, 1], F32)
            nc.vector.reciprocal(rinv, rows)
            samp = const.tile([EOUT, EINP], F32)
            nc.vector.tensor_scalar_mul(samp, ez, rinv)

            # samples^T chunks for use as matmul stationary operand
            sampT = const.tile([128, NLIB, EOUT], F32R)
            for c in range(NLIB):
                pt = ps_t.tile([128, 128], F32, tag="pt")
                nc.tensor.transpose(pt, samp[:, c * 128:(c + 1) * 128], ident)
                nc.vector.tensor_copy(sampT[:, c], pt)

            # A_tilde^T (stationary operand for D^T matmul)
            atT = const.tile([R, R], F32R)
            ptA = ps_t.tile([128, 128], F32, tag="pt")
            nc.tensor.transpose(ptA[:R, :R], at_sb, ident[:R, :R])
            nc.vector.tensor_copy(atT, ptA[:R, :R])

            # ---- phase X: stream all X chunks on both HWDGE queues
            xacc = ps_acc.tile([R, BS], F32, tag="acc")
            for c in range(KC):
                xtile = xin.tile([128, BS], F32R, tag="xin")
                eng = nc.sync if c % 2 == 0 else nc.scalar
                eng.dma_start(
                    out=xtile, in_=xt[c * 128:(c + 1) * 128, :].bitcast(F32R))
                nc.tensor.matmul(
                    xacc, u_sb[:, c], xtile[:],
                    start=(c == 0), stop=(c == KC - 1))

            # affine (x + 1)/2 applied during PSUM->SBUF copy
            xt_T = const.tile([R, BS], F32R)
            nc.scalar.activation(out=xt_T, in_=xacc, func=AF.Copy, scale=0.5, bias=0.5)

            # Xh^T = phi^T @ Xt^T, then library [id, square, sin]
            libT = const.tile([128, NLIB, BS], F32R)
            for c in range(2):
                xh_ps = ps_acc.tile([128, BS], F32, tag="acc")
                nc.tensor.matmul(xh_ps, phi_sb[:, c * 128:(c + 1) * 128], xt_T[:],
                                 start=True, stop=True)
                nc.vector.tensor_copy(libT[:, c], xh_ps)
                nc.vector.tensor_mul(libT[:, 2 + c], libT[:, c], libT[:, c])
                nc.scalar.activation(out=libT[:, 4 + c], in_=xh_ps, func=AF.Sin)

            # A^T = samples @ lib^T (accumulate over 6 library chunks)
            a_ps = ps_acc.tile([128, BS], F32, tag="acc")
            for c in range(NLIB):
                nc.tensor.matmul(a_ps, sampT[:, c], libT[:, c],
                                 start=(c == 0), stop=(c == NLIB - 1))
            aT = const.tile([128, BS], F32)
            nc.vector.tensor_copy(aT, a_ps)

            # D^T partial = A_tilde @ Xt^T (subtract Yt^T later)
            d_ps = ps_acc.tile([R, BS], F32, tag="acc")
            nc.tensor.matmul(d_ps, atT[:], xt_T[:], start=True, stop=True)

            # A natural + X-only contractions (G0, ATB) — overlap with Y stream
            a_nat = const.tile([128, BC, 128], F32)
            for bc in range(BC):
                pa = ps_t.tile([128, 128], F32, tag="pt")
                nc.tensor.transpose(pa, aT[:, bc * 128:(bc + 1) * 128], ident)
                nc.vector.tensor_copy(a_nat[:, bc], pa)

            g_ps = ps_acc.tile([128, 128], F32, tag="acc")
            for bc in range(BC):
                nc.tensor.matmul(g_ps, a_nat[:, bc], a_nat[:, bc],
                                 start=(bc == 0), stop=(bc == BC - 1))
            g_sb = const.tile([128, 128], F32)
            nc.vector.tensor_copy(g_sb, g_ps)
            nc.sync.dma_start(out=g0_o[:, :], in_=g_sb)

            atb_ps = ps_acc.tile([128, R], F32, tag="acc")
            for bc in range(BC):
                nc.tensor.matmul(atb_ps, a_nat[:, bc], lhs_sb[:, bc],
                                 start=(bc == 0), stop=(bc == BC - 1))
            atb_sb = const.tile([128, R], F32)
            nc.vector.tensor_copy(atb_sb, atb_ps)
            nc.sync.dma_start(out=atb_o[:, :], in_=atb_sb)

            # ---- phase Y: stream all Y chunks
            yacc = ps_acc.tile([R, BS], F32, tag="acc")
            for c in range(KC):
                ytile = xin.tile([128, BS], F32R, tag="xin")
                eng = nc.sync if c % 2 == 0 else nc.scalar
                eng.dma_start(
                    out=ytile, in_=yt[c * 128:(c + 1) * 128, :].bitcast(F32R))
                nc.tensor.matmul(
                    yacc, u_sb[:, c], ytile[:],
                    start=(c == 0), stop=(c == KC - 1))

            yt_T = const.tile([R, BS], F32)
            nc.scalar.activation(out=yt_T, in_=yacc, func=AF.Copy, scale=0.5, bias=0.5)

            # yn = ||Yt||^2 (per-partition partial sums)
            sq_y = const.tile([R, BS], F32)
            yn_sb = const.tile([R, 1], F32)
            nc.scalar.activation(out=sq_y, in_=yt_T, func=AF.Square, accum_out=yn_sb)
            nc.scalar.dma_start(out=yn_o[:, :], in_=yn_sb)

            # D^T = d_ps - Yt^T, dn = ||D||^2
            dT = const.tile([R, BS], F32)
            nc.vector.tensor_sub(dT, d_ps, yt_T)
            sq_d = const.tile([R, BS], F32)
            dn_sb = const.tile([R, 1], F32)
            nc.scalar.activation(out=sq_d, in_=dT, func=AF.Square, accum_out=dn_sb)
            nc.scalar.dma_start(out=dn_o[:, :], in_=dn_sb)

            # D natural + ATD
            d_nat = const.tile([128, BC, R], F32)
            for bc in range(BC):
                pd = ps_t.tile([128, 128], F32, tag="pt")
                nc.tensor.transpose(pd[:, :R], dT[:, bc * 128:(bc + 1) * 128],
                                    ident[:R, :R])
                nc.vector.tensor_copy(d_nat[:, bc], pd[:, :R])

            atd_ps = ps_acc.tile([128, R], F32, tag="acc")
            for bc in range(BC):
                nc.tensor.matmul(atd_ps, a_nat[:, bc], d_nat[:, bc],
                                 start=(bc == 0), stop=(bc == BC - 1))
            atd_sb = const.tile([128, R], F32)
            nc.vector.tensor_copy(atd_sb, atd_ps)
            nc.sync.dma_start(out=atd_o[:, :], in_=atd_sb)

    nc.compile()
    return nc


_cache = {}


def kernel(X_batch_t, Y_batch_t, lhs_mat_batch_t, temperature_spt, logits,
           uniform_noise, U_l, phi_mat, A_tilde, lam_vec,
           _trace=False, _trace_cores=None):
    if "nc" not in _cache:
        _cache["nc"] = build_nc()
    nc = _cache["nc"]

    X = np.asarray(X_batch_t, dtype=np.float32)
    Y = np.asarray(Y_batch_t, dtype=np.float32)
    lhs = np.asarray(lhs_mat_batch_t, dtype=np.float32)
    U = np.asarray(U_l, dtype=np.float32)
    U = np.ascontiguousarray(U.reshape(KC, 128, R).transpose(1, 0, 2).reshape(128, KC * R))
    phim = np.ascontiguousarray(np.asarray(phi_mat, dtype=np.float32))
    atil = np.ascontiguousarray(np.asarray(A_tilde, dtype=np.float32))
    logits_np = np.ascontiguousarray(np.asarray(logits, dtype=np.float32))
    noise_np = np.ascontiguousarray(np.asarray(uniform_noise, dtype=np.float32))
    tval = np.float32(np.asarray(temperature_spt).reshape(-1)[0])
    temp = np.maximum(np.float32(0.1), tval * np.float32(0.99))
    itemp = np.full((EOUT, 1), np.float32(1.0) / temp, dtype=np.float32)

    in_maps = []
    for c in range(N_CORES):
        sl = slice(c * BS, (c + 1) * BS)
        in_maps.append({
            "xt": np.ascontiguousarray(X[sl].T),
            "yt": np.ascontiguousarray(Y[sl].T),
            "lhs": np.ascontiguousarray(
                lhs[sl].reshape(BC, 128, R).transpose(1, 0, 2).reshape(128, BC * R)),
            "ul": U, "phim": phim, "atil": atil,
            "logits": logits_np, "noise": noise_np, "itemp": itemp,
        })

    kw = {}
    if _trace:
        kw["trace"] = True
        kw["trace_cores"] = _trace_cores or list(range(N_CORES))
    res = run_bass_kernel_spmd(nc, in_maps, core_ids=list(range(N_CORES)), **kw)
    kernel.last = res
    results = res.results

    G0 = np.zeros((EOUT, EOUT), np.float64)
    ATB = np.zeros((EOUT, R), np.float64)
    ATD = np.zeros((EOUT, R), np.float64)
    dn = 0.0
    yn = 0.0
    for r in results:
        G0 += r["g0"].astype(np.float64)
        ATB += r["atb"].astype(np.float64)
        ATD += r["atd"].astype(np.float64)
        dn += float(r["dn"].astype(np.float64).sum())
        yn += float(r["yn"].astype(np.float64).sum())

    lamv = np.asarray(lam_vec, dtype=np.float64).reshape(-1)
    eye = np.eye(EOUT)
    errs = np.empty(len(lamv))
    Ws = []
    for i, lam in enumerate(lamv):
        W = np.linalg.solve(G0 + lam * eye, ATB)
        num = dn + 2.0 * float(np.sum(W * ATD)) + float(np.sum(W * (G0 @ W)))
        errs[i] = np.sqrt(max(num, 0.0) / yn)
        Ws.append(W)
    iopt = int(np.nanargmin(errs))
    phi_bar = np.ascontiguousarray(Ws[iopt].T).astype(np.float32)
    selected_idx = np.argmax(logits_np, axis=-1).astype(np.int32)
    return (selected_idx, np.float32(errs[iopt]), phi_bar, np.float32(temp))


kernel.last = None
